# revision 10
# baseline (speedup 1.0000x reference)
"""Trainium2 Bass kernel: attention-GRU decoder (nn_Attention_45792941310497).

Data-parallel over batch: B=512 -> 64 per core on 8 NeuronCores.
Per core layout:
  bht  [T=128 part, b*512+d]  bf16  - batch_H^T, resident, context matmul stationary
  bhd  [d, b*128+t]           bf16  - transient, for H_proj compute
  Hp   2 x [128(h), b*128+t]  bf16  - H_proj^T, resident
  u[h] [128(h), (b, c, t)]    bf16  - tanh workspace per half, c-interleaved

26 serial steps; the two 32-batch halves are software-pipelined across
steps: per step the issue order is attn(h0,s) gru(h0,s) attn(h1,s)
prep(h0,s+1) gru(h1,s) prep(h1,s+1), where prep = h_proj + per-(b,c)
tensor_scalar adds (DVE 4x mode) + tanh.  Softmax is done in [t, b]
layout: partition-sum via ones-matmul (replicated rows), reciprocal and
alpha elementwise on the replicated tiles -- no PE transposes.
"""

import os
import sys

sys.path.insert(0, "/opt/trn_rl_repo")

import numpy as np
import ml_dtypes

BF16 = ml_dtypes.bfloat16

B, T, D, HID, C = 512, 128, 512, 256, 96
G = 3 * HID  # 768
NSTEP = int(os.environ.get("ATT_NSTEPS", "26"))
NCORES = 8
BL = B // NCORES  # 64
HB = BL // 2      # 32 per half
QB = HB // 2      # 16 per quarter

_CACHE = {}
LAST_RESULT = None


def _build():
    from concourse import bacc, tile, mybir
    from concourse.bass import MemorySpace

    dt = mybir.dt
    AF = mybir.ActivationFunctionType

    nc = bacc.Bacc(None, target_bir_lowering=False)

    # ---------------- DRAM I/O ----------------
    bht_d = nc.dram_tensor("bht", [128, BL * D], dt.bfloat16, kind="ExternalInput")
    bhd_d = nc.dram_tensor("bhd", [D, BL * T], dt.bfloat16, kind="ExternalInput")
    wi2hT_d = nc.dram_tensor("wi2hT", [D, HID], dt.bfloat16, kind="ExternalInput")
    wh2hT_d = nc.dram_tensor("wh2hT", [HID, HID], dt.bfloat16, kind="ExternalInput")
    bh2h_d = nc.dram_tensor("bh2h", [128, 2], dt.float32, kind="ExternalInput")
    wsc_d = nc.dram_tensor("wsc", [128, 2], dt.bfloat16, kind="ExternalInput")
    wihcT_d = nc.dram_tensor("wihcT", [D, G], dt.bfloat16, kind="ExternalInput")
    whhT_d = nc.dram_tensor("whhT", [HID, G], dt.bfloat16, kind="ExternalInput")
    goh_d = nc.dram_tensor("goh", [128, NSTEP * 6 * BL], dt.bfloat16, kind="ExternalInput")
    wgenT_d = nc.dram_tensor("wgenT", [HID, C], dt.bfloat16, kind="ExternalInput")
    bgen_d = nc.dram_tensor("bgen", [C, BL], dt.float32, kind="ExternalInput")
    ident_d = nc.dram_tensor("ident", [128, 128], dt.bfloat16, kind="ExternalInput")
    ones_d = nc.dram_tensor("ones", [128, 128], dt.bfloat16, kind="ExternalInput")
    out_d = nc.dram_tensor("out", [C, NSTEP * BL], dt.float32, kind="ExternalOutput")

    with tile.TileContext(nc) as tc:
        with tc.tile_pool(name="res", bufs=1) as res:
            # resident tiles
            bht = res.tile([128, BL * D], dt.bfloat16, tag="bht", name="bht")
            hp = [res.tile([128, BL * T], dt.bfloat16, tag=f"hp{c}", name=f"hp{c}") for c in range(2)]
            wh2hT = [res.tile([128, HID], dt.bfloat16, tag=f"wh2hT{k}", name=f"wh2hT{k}") for k in range(2)]
            bh2h = res.tile([128, 2], dt.float32, tag="bh2h", name="bh2h")
            wsc = res.tile([128, 2], dt.bfloat16, tag="wsc", name="wsc")
            wihcT = [res.tile([128, G], dt.bfloat16, tag=f"wihcT{k}", name=f"wihcT{k}") for k in range(4)]
            whhT = [res.tile([128, G], dt.bfloat16, tag=f"whhT{k}", name=f"whhT{k}") for k in range(2)]
            goh = res.tile([128, NSTEP * 6 * BL], dt.bfloat16, tag="goh", name="goh")
            wgenT = [res.tile([128, C], dt.bfloat16, tag=f"wgenT{k}", name=f"wgenT{k}") for k in range(2)]
            bgen = res.tile([C, BL], dt.float32, tag="bgen", name="bgen")
            ident = res.tile([128, 128], dt.bfloat16, tag="ident", name="ident")
            ones = res.tile([128, 128], dt.bfloat16, tag="ones", name="ones")
            pacc = res.tile([C, NSTEP * BL], dt.float32, tag="pacc", name="pacc")

            # step-phase weights (small, needed first)
            for k in range(2):
                nc.sync.dma_start(wh2hT[k][:], wh2hT_d[k * 128:(k + 1) * 128, :])
            nc.sync.dma_start(bh2h[:], bh2h_d[:])
            nc.sync.dma_start(wsc[:], wsc_d[:])
            nc.sync.dma_start(ident[:], ident_d[:])
            nc.sync.dma_start(ones[:], ones_d[:])

            # ---------- setup: H_proj = batch_H @ W_i2h.T  (as Hp[c][h, b*T+t]) ----------
            with (
                tc.tile_pool(name="setup", bufs=1) as sp,
                tc.tile_pool(name="setup_ps", bufs=4, space=MemorySpace.PSUM) as spp,
            ):
                # warm the ACT table set (exp/tanh) during setup DMA
                dummy = sp.tile([128, 2], dt.float32, tag="dummy", name="dummy")
                nc.vector.memset(dummy[:], 0.0)
                nc.scalar.activation(dummy[:], dummy[:], AF.Tanh)

                bhd = [sp.tile([128, BL * T], dt.bfloat16, tag=f"bhd{k}", name=f"bhd{k}") for k in range(4)]
                wi2hT = [sp.tile([128, HID], dt.bfloat16, tag=f"wi2hT{k}", name=f"wi2hT{k}") for k in range(4)]
                for k in range(4):
                    nc.sync.dma_start(wi2hT[k][:], wi2hT_d[k * 128:(k + 1) * 128, :])
                    for j in range(2):
                        sl = slice(j * BL * T // 2, (j + 1) * BL * T // 2)
                        nc.sync.dma_start(bhd[k][:, sl], bhd_d[k * 128:(k + 1) * 128, sl])

                # remaining big inputs, enqueued after the setup-critical ones
                for i in range(8):
                    sl = slice(i * BL * D // 8, (i + 1) * BL * D // 8)
                    nc.sync.dma_start(bht[:, sl], bht_d[:, sl])
                nc.sync.dma_start(goh[:], goh_d[:])
                for k in range(4):
                    nc.sync.dma_start(wihcT[k][:], wihcT_d[k * 128:(k + 1) * 128, :])
                for k in range(2):
                    nc.sync.dma_start(whhT[k][:], whhT_d[k * 128:(k + 1) * 128, :])
                    nc.sync.dma_start(wgenT[k][:], wgenT_d[k * 128:(k + 1) * 128, :])
                nc.sync.dma_start(bgen[:], bgen_d[:])

                for m in range(2):
                    for nb in range(BL * T // 512):
                        ps = spp.tile([128, 512], dt.float32, tag="hps", name="hps")
                        for k in range(4):
                            nc.tensor.matmul(
                                ps[:],
                                wi2hT[k][:, m * 128:(m + 1) * 128],
                                bhd[k][:, nb * 512:(nb + 1) * 512],
                                start=(k == 0),
                                stop=(k == 3),
                            )
                        nc.vector.tensor_scalar_add(
                            hp[m][:, nb * 512:(nb + 1) * 512], ps[:], bh2h[:, m:m + 1]
                        )

            # ---------- recurrent steps ----------
            with (
                tc.tile_pool(name="work", bufs=1) as wk,
                tc.tile_pool(name="small", bufs=2) as sm,
                tc.tile_pool(name="hidp", bufs=2) as hidp,
                tc.tile_pool(name="ps", bufs=1, space=MemorySpace.PSUM) as pp,
            ):
                u = [None, None]
                hT = [None, None]
                hTb = [None, None]
                for h in range(2):
                    t_f = hidp.tile([128, 2 * HB], dt.float32, tag=f"hT{h}", name=f"hT{h}")
                    t_b = hidp.tile([128, 2 * HB], dt.bfloat16, tag=f"hTb{h}", name=f"hTb{h}")
                    nc.vector.memset(t_f[:], 0.0)
                    nc.vector.memset(t_b[:], 0.0)
                    hT[h] = t_f
                    hTb[h] = t_b

                goh_v = goh[:].rearrange("p (s c b) -> p s c b", c=6, b=BL)
                alphaT = [None, None]

                hp_ps_l = [None, None]

                def prepH(h):
                    # h_proj = W_h2h^T h  (b_h2h is folded into Hp)
                    hp_ps = pp.tile([128, 2 * HB], dt.float32, tag=f"hp_ps{h}", name=f"hp_ps{h}")
                    hp_ps_l[h] = hp_ps
                    for c in range(2):
                        for k in range(2):
                            nc.tensor.matmul(
                                hp_ps[:, c * HB:(c + 1) * HB],
                                wh2hT[k][:, c * 128:(c + 1) * 128],
                                hTb[h][:, k * HB:(k + 1) * HB],
                                start=(c == 0 and k == 0),
                                stop=(c == 1 and k == 1),
                                skip_group_check=True,
                            )
                    u[h] = wk.tile([128, HB * 2 * T], dt.bfloat16, tag=f"u{h}", name=f"u{h}")

                def prepAdd(h, piece):
                    # u[b, c, t] = Hp[c][b, t] + h_proj[c, b]  (broadcast over t)
                    hp_ps = hp_ps_l[h]
                    u_v = u[h][:].rearrange("p (b ct) -> p b ct", ct=2 * T)
                    bsl = slice(h * HB + piece * QB, h * HB + (piece + 1) * QB)
                    osl = slice(piece * QB, (piece + 1) * QB)
                    for c in range(2):
                        nc.vector.tensor_add(
                            u_v[:, osl, c * T:(c + 1) * T],
                            hp[c][:].rearrange("p (b t) -> p b t", t=T)[:, bsl, :],
                            hp_ps[:, c * HB + piece * QB:c * HB + (piece + 1) * QB].to_broadcast((128, QB, T)),
                        )

                def prepTanh(h, piece):
                    sl = slice(piece * QB * 2 * T, (piece + 1) * QB * 2 * T)
                    nc.scalar.activation(u[h][:, sl], u[h][:, sl], AF.Tanh)

                def attn(h):
                    # e_ss packs e (cols 0:2QB) and sum-of-exp (cols 2QB:4QB)
                    e_ss = pp.tile([128, 4 * QB], dt.float32, tag=f"e_ss{h}", name=f"e_ss{h}")
                    for piece in range(2):
                        for c in range(2):
                            for b in range(QB):
                                bl = piece * QB + b
                                nc.tensor.matmul(
                                    e_ss[:, bl:bl + 1],
                                    u[h][:, (bl * 2 + c) * T:(bl * 2 + c + 1) * T],
                                    wsc[:, c:c + 1],
                                    start=(c == 0 and b == 0),
                                    stop=(c == 1 and b == QB - 1),
                                    skip_group_check=True,
                                )
                    expe = sm.tile([128, 2 * QB], dt.bfloat16, tag=f"expe{h}", name=f"expe{h}")
                    nc.scalar.activation(expe[:], e_ss[:, 0:2 * QB], AF.Exp)
                    nc.tensor.matmul(
                        e_ss[:, 2 * QB:4 * QB],
                        ones[:],
                        expe[:],
                        start=True,
                        stop=True,
                        skip_group_check=True,
                    )
                    rs = sm.tile([128, 2 * QB], dt.float32, tag=f"rs{h}", name=f"rs{h}")
                    nc.vector.reciprocal(rs[:], e_ss[:, 2 * QB:4 * QB])
                    al = sm.tile([128, 2 * QB], dt.bfloat16, tag=f"al{h}", name=f"al{h}")
                    nc.vector.tensor_mul(al[:], expe[:], rs[:])
                    alphaT[h] = al
                    ctxT = [None, None]
                    for piece in range(2):
                        ctx_ps = pp.tile([128, 4 * QB], dt.float32, tag="ctx_ps", name=f"ctx_ps{h}{piece}", bufs=2)
                        for ck in range(4):
                            for b in range(QB):
                                gb = h * HB + piece * QB + b
                                nc.tensor.matmul(
                                    ctx_ps[:, ck * QB + b:ck * QB + b + 1],
                                    bht[:, gb * D + ck * 128:gb * D + (ck + 1) * 128],
                                    al[:, piece * QB + b:piece * QB + b + 1],
                                    start=(ck == 0 and b == 0),
                                    stop=(ck == 3 and b == QB - 1),
                                    skip_group_check=True,
                                )
                        ct = sm.tile([128, 4 * QB], dt.bfloat16, tag=f"ctxT{h}{piece}", name=f"ctxT{h}{piece}")
                        nc.scalar.copy(ct[:], ctx_ps[:])
                        ctxT[piece] = ct
                    return ctxT

                def gru(h, s, ctxT):
                    # gi cols 0:6HB, ghn cols 6HB:8HB packed in one psum tile
                    gi_ps = pp.tile([128, 8 * HB], dt.float32, tag="gi_ps", name="gi_ps")
                    for ck in range(4):
                        for m in range(6):
                            for piece in range(2):
                                nc.tensor.matmul(
                                    gi_ps[:, m * HB + piece * QB:m * HB + (piece + 1) * QB],
                                    wihcT[ck][:, m * 128:(m + 1) * 128],
                                    ctxT[piece][:, ck * QB:(ck + 1) * QB],
                                    start=(ck == 0 and m == 0 and piece == 0),
                                    stop=False,
                                    skip_group_check=True,
                                )
                    for m in range(6):
                        nc.tensor.matmul(
                            gi_ps[:, m * HB:(m + 1) * HB],
                            ident[:],
                            goh_v[:, s, m, h * HB:(h + 1) * HB],
                            start=False,
                            stop=False,
                            skip_group_check=True,
                        )
                    for k in range(2):
                        for m in range(4):
                            nc.tensor.matmul(
                                gi_ps[:, m * HB:(m + 1) * HB],
                                whhT[k][:, m * 128:(m + 1) * 128],
                                hTb[h][:, k * HB:(k + 1) * HB],
                                start=False,
                                stop=(k == 1 and m == 3),
                                skip_group_check=True,
                            )
                    for k in range(2):
                        for m in range(4, 6):
                            nc.tensor.matmul(
                                gi_ps[:, (m + 2) * HB:(m + 3) * HB],
                                whhT[k][:, m * 128:(m + 1) * 128],
                                hTb[h][:, k * HB:(k + 1) * HB],
                                start=(k == 0 and m == 4),
                                stop=(k == 1 and m == 5),
                                skip_group_check=True,
                            )
                    # sigmoid(x) = 0.5*tanh(x/2)+0.5; whhT n-cols pre-halved on host
                    trz = sm.tile([128, 4 * HB], dt.float32, tag="trz", name="trz")
                    nc.scalar.activation(trz[:], gi_ps[:, 0:4 * HB], AF.Tanh, scale=0.5)
                    rh = sm.tile([128, 2 * HB], dt.float32, tag="rh", name="rh")
                    nc.vector.scalar_tensor_tensor(
                        rh[:], trz[:, 0:2 * HB], 1.0, gi_ps[:, 6 * HB:8 * HB],
                        op0=mybir.AluOpType.add, op1=mybir.AluOpType.mult,
                    )
                    pre_n = sm.tile([128, 2 * HB], dt.float32, tag="pre_n", name="pre_n")
                    nc.vector.tensor_add(pre_n[:], gi_ps[:, 4 * HB:6 * HB], rh[:])
                    nt = sm.tile([128, 2 * HB], dt.float32, tag="nt", name="nt")
                    nc.scalar.activation(nt[:], pre_n[:], AF.Tanh)
                    dmn = sm.tile([128, 2 * HB], dt.float32, tag="dmn", name="dmn")
                    nc.vector.tensor_sub(dmn[:], hT[h][:], nt[:])
                    zd = sm.tile([128, 2 * HB], dt.float32, tag="zd", name="zd")
                    nc.vector.scalar_tensor_tensor(
                        zd[:], trz[:, 2 * HB:4 * HB], 1.0, dmn[:],
                        op0=mybir.AluOpType.add, op1=mybir.AluOpType.mult,
                    )
                    nh = hidp.tile([128, 2 * HB], dt.float32, tag=f"hT{h}", name=f"hT{h}")
                    nc.vector.scalar_tensor_tensor(
                        nh[:], zd[:], 0.5, nt[:],
                        op0=mybir.AluOpType.mult, op1=mybir.AluOpType.add,
                    )
                    nhb = hidp.tile([128, 2 * HB], dt.bfloat16, tag=f"hTb{h}", name=f"hTb{h}")
                    nc.vector.tensor_copy(nhb[:], nh[:])
                    hT[h] = nh
                    hTb[h] = nhb

                    pr_ps = pp.tile([C, HB], dt.float32, tag="pr_ps", name="pr_ps")
                    for k in range(2):
                        nc.tensor.matmul(
                            pr_ps[:],
                            wgenT[k][:],
                            nhb[:, k * HB:(k + 1) * HB],
                            start=(k == 0),
                            stop=(k == 1),
                            skip_group_check=True,
                        )
                    nc.vector.tensor_add(
                        pacc[:, s * BL + h * HB:s * BL + (h + 1) * HB],
                        pr_ps[:],
                        bgen[:, 0:HB],
                    )

                for h in range(2):
                    prepH(h)
                    for piece in range(2):
                        prepAdd(h, piece)
                        prepTanh(h, piece)
                for s in range(NSTEP):
                    last = s + 1 >= NSTEP
                    ctx0 = attn(0)
                    gru(0, s, ctx0)
                    if not last:
                        prepH(0)
                        prepAdd(0, 0)
                        prepTanh(0, 0)
                    ctx1 = attn(1)
                    if not last:
                        prepAdd(0, 1)
                        prepTanh(0, 1)
                    gru(1, s, ctx1)
                    if not last:
                        prepH(1)
                        for piece in range(2):
                            prepAdd(1, piece)
                            prepTanh(1, piece)

            for j in range(4):
                sl = slice(j * NSTEP * BL // 4, (j + 1) * NSTEP * BL // 4)
                nc.sync.dma_start(out_d[:, sl], pacc[:, sl])

    nc.compile()
    return nc


def kernel(**inputs):
    global LAST_RESULT
    from concourse.bass_utils import run_bass_kernel_spmd

    if "nc" not in _CACHE:
        _CACHE["nc"] = _build()
    nc = _CACHE["nc"]

    batch_H = np.asarray(inputs["batch_H"], dtype=np.float32)
    text = np.asarray(inputs["text"])
    W_i2h = np.asarray(inputs["W_i2h"], dtype=np.float32)
    W_h2h = np.asarray(inputs["W_h2h"], dtype=np.float32)
    b_h2h = np.asarray(inputs["b_h2h"], dtype=np.float32)
    W_score = np.asarray(inputs["W_score"], dtype=np.float32)
    W_ih = np.asarray(inputs["W_ih"], dtype=np.float32)
    W_hh = np.asarray(inputs["W_hh"], dtype=np.float32)
    b_ih = np.asarray(inputs["b_ih"], dtype=np.float32)
    b_hh = np.asarray(inputs["b_hh"], dtype=np.float32)
    W_gen = np.asarray(inputs["W_gen"], dtype=np.float32)
    b_gen = np.asarray(inputs["b_gen"], dtype=np.float32)

    shared = {
        "wi2hT": np.ascontiguousarray(W_i2h.T).astype(BF16),
        "wh2hT": np.ascontiguousarray(W_h2h.T).astype(BF16),
        "bh2h": np.ascontiguousarray(b_h2h.reshape(2, 128).T).astype(np.float32),
        "wsc": np.ascontiguousarray(W_score[0].reshape(2, 128).T).astype(BF16),
        "wihcT": np.ascontiguousarray(W_ih[:, :D].T).astype(BF16),
        "whhT": np.ascontiguousarray(W_hh.T * np.concatenate([np.ones(512, np.float32), np.full(256, 0.5, np.float32)])[None, :]).astype(BF16),
        "wgenT": np.ascontiguousarray(W_gen.T).astype(BF16),
        "bgen": np.ascontiguousarray(np.tile(b_gen[:, None], (1, BL))).astype(np.float32),
        "ident": np.eye(128, dtype=np.float32).astype(BF16),
        "ones": np.ones((128, 128), dtype=np.float32).astype(BF16),
    }

    Eoh = W_ih[:, D:]  # [768, 96]
    bias = (b_ih + b_hh)[:, None, None]  # folded; b_hh==0 in this problem

    in_maps = []
    for ci in range(NCORES):
        sh = batch_H[ci * BL:(ci + 1) * BL]  # [64, 128, 512]
        tx = np.asarray(text[ci * BL:(ci + 1) * BL, :NSTEP], dtype=np.int64)  # [64, S]
        A = Eoh[:, tx] + bias  # [768, 64, S]
        gohm = (
            A.reshape(6, 128, BL, NSTEP)
            .transpose(1, 3, 0, 2)
            .reshape(128, NSTEP * 6 * BL)
        )
        m = dict(shared)
        m["bht"] = np.ascontiguousarray(sh.transpose(1, 0, 2).reshape(128, BL * D)).astype(BF16)
        m["bhd"] = np.ascontiguousarray(sh.transpose(2, 0, 1).reshape(D, BL * T)).astype(BF16)
        m["goh"] = np.ascontiguousarray(gohm).astype(BF16)
        in_maps.append(m)

    trace = bool(os.environ.get("ATT_TRACE"))
    res = run_bass_kernel_spmd(nc, in_maps, list(range(NCORES)), trace=trace)
    LAST_RESULT = res

    outs = []
    for r in res.results:
        o = r["out"].reshape(C, NSTEP, BL).transpose(2, 1, 0)  # [64, S, 96]
        outs.append(o)
    return np.ascontiguousarray(np.concatenate(outs, axis=0)).astype(np.float32)


# revision 11
# speedup vs baseline: 1.0023x; 1.0023x over previous
"""Trainium2 Bass kernel: attention-GRU decoder (nn_Attention_45792941310497).

Data-parallel over batch: B=512 -> 64 per core on 8 NeuronCores.
Per core layout:
  bht  [T=128 part, b*512+d]  bf16  - batch_H^T, resident, context matmul stationary
  bhd  [d, b*128+t]           bf16  - transient, for H_proj compute
  Hp   2 x [128(h), b*128+t]  bf16  - H_proj^T, resident
  u[h] [128(h), (b, c, t)]    bf16  - tanh workspace per half, c-interleaved

26 serial steps; the two 32-batch halves are software-pipelined across
steps: per step the issue order is attn(h0,s) gru(h0,s) attn(h1,s)
prep(h0,s+1) gru(h1,s) prep(h1,s+1), where prep = h_proj + per-(b,c)
tensor_scalar adds (DVE 4x mode) + tanh.  Softmax is done in [t, b]
layout: partition-sum via ones-matmul (replicated rows), reciprocal and
alpha elementwise on the replicated tiles -- no PE transposes.
"""

import os
import sys

sys.path.insert(0, "/opt/trn_rl_repo")

import numpy as np
import ml_dtypes

BF16 = ml_dtypes.bfloat16

B, T, D, HID, C = 512, 128, 512, 256, 96
G = 3 * HID  # 768
NSTEP = int(os.environ.get("ATT_NSTEPS", "26"))
NCORES = 8
BL = B // NCORES  # 64
HB = BL // 2      # 32 per half
QB = HB // 2      # 16 per quarter

_CACHE = {}
LAST_RESULT = None


def _build():
    from concourse import bacc, tile, mybir
    from concourse.bass import MemorySpace

    dt = mybir.dt
    AF = mybir.ActivationFunctionType

    nc = bacc.Bacc(None, target_bir_lowering=False)

    # ---------------- DRAM I/O ----------------
    bht_d = nc.dram_tensor("bht", [128, BL * D], dt.bfloat16, kind="ExternalInput")
    bhd_d = nc.dram_tensor("bhd", [D, BL * T], dt.bfloat16, kind="ExternalInput")
    wi2hT_d = nc.dram_tensor("wi2hT", [D, HID], dt.bfloat16, kind="ExternalInput")
    wh2hT_d = nc.dram_tensor("wh2hT", [HID, HID], dt.bfloat16, kind="ExternalInput")
    bh2h_d = nc.dram_tensor("bh2h", [128, 2], dt.float32, kind="ExternalInput")
    wsc_d = nc.dram_tensor("wsc", [128, 2], dt.bfloat16, kind="ExternalInput")
    wihcT_d = nc.dram_tensor("wihcT", [D, G], dt.bfloat16, kind="ExternalInput")
    whhT_d = nc.dram_tensor("whhT", [HID, G], dt.bfloat16, kind="ExternalInput")
    goh_d = nc.dram_tensor("goh", [128, NSTEP * 6 * BL], dt.bfloat16, kind="ExternalInput")
    wgenT_d = nc.dram_tensor("wgenT", [HID, C], dt.bfloat16, kind="ExternalInput")
    bgen_d = nc.dram_tensor("bgen", [C, BL], dt.float32, kind="ExternalInput")
    ident_d = nc.dram_tensor("ident", [128, 128], dt.bfloat16, kind="ExternalInput")
    ones_d = nc.dram_tensor("ones", [128, 128], dt.bfloat16, kind="ExternalInput")
    out_d = nc.dram_tensor("out", [C, NSTEP * BL], dt.float32, kind="ExternalOutput")

    with tile.TileContext(nc) as tc:
        with tc.tile_pool(name="res", bufs=1) as res:
            # resident tiles
            bht = res.tile([128, BL * D], dt.bfloat16, tag="bht", name="bht")
            hp = [res.tile([128, BL * T], dt.bfloat16, tag=f"hp{c}", name=f"hp{c}") for c in range(2)]
            wh2hT = [res.tile([128, HID], dt.bfloat16, tag=f"wh2hT{k}", name=f"wh2hT{k}") for k in range(2)]
            bh2h = res.tile([128, 2], dt.float32, tag="bh2h", name="bh2h")
            wsc = res.tile([128, 2], dt.bfloat16, tag="wsc", name="wsc")
            wihcT = [res.tile([128, G], dt.bfloat16, tag=f"wihcT{k}", name=f"wihcT{k}") for k in range(4)]
            whhT = [res.tile([128, G], dt.bfloat16, tag=f"whhT{k}", name=f"whhT{k}") for k in range(2)]
            goh = res.tile([128, NSTEP * 6 * BL], dt.bfloat16, tag="goh", name="goh")
            wgenT = [res.tile([128, C], dt.bfloat16, tag=f"wgenT{k}", name=f"wgenT{k}") for k in range(2)]
            bgen = res.tile([C, BL], dt.float32, tag="bgen", name="bgen")
            ident = res.tile([128, 128], dt.bfloat16, tag="ident", name="ident")
            ones = res.tile([128, 128], dt.bfloat16, tag="ones", name="ones")
            pacc = res.tile([C, NSTEP * BL], dt.float32, tag="pacc", name="pacc")

            # step-phase weights (small, needed first)
            for k in range(2):
                nc.sync.dma_start(wh2hT[k][:], wh2hT_d[k * 128:(k + 1) * 128, :])
            nc.sync.dma_start(bh2h[:], bh2h_d[:])
            nc.sync.dma_start(wsc[:], wsc_d[:])
            nc.sync.dma_start(ident[:], ident_d[:])
            nc.sync.dma_start(ones[:], ones_d[:])

            # ---------- setup: H_proj = batch_H @ W_i2h.T  (as Hp[c][h, b*T+t]) ----------
            with (
                tc.tile_pool(name="setup", bufs=1) as sp,
                tc.tile_pool(name="setup_ps", bufs=4, space=MemorySpace.PSUM) as spp,
            ):
                # warm the ACT table set (exp/tanh) during setup DMA
                dummy = sp.tile([128, 2], dt.float32, tag="dummy", name="dummy")
                nc.vector.memset(dummy[:], 0.0)
                nc.scalar.activation(dummy[:], dummy[:], AF.Tanh)

                bhd = [sp.tile([128, BL * T], dt.bfloat16, tag=f"bhd{k}", name=f"bhd{k}") for k in range(4)]
                wi2hT = [sp.tile([128, HID], dt.bfloat16, tag=f"wi2hT{k}", name=f"wi2hT{k}") for k in range(4)]
                for k in range(4):
                    nc.sync.dma_start(wi2hT[k][:], wi2hT_d[k * 128:(k + 1) * 128, :])
                    for j in range(2):
                        sl = slice(j * BL * T // 2, (j + 1) * BL * T // 2)
                        nc.sync.dma_start(bhd[k][:, sl], bhd_d[k * 128:(k + 1) * 128, sl])

                # remaining big inputs, enqueued after the setup-critical ones
                for i in range(8):
                    sl = slice(i * BL * D // 8, (i + 1) * BL * D // 8)
                    nc.sync.dma_start(bht[:, sl], bht_d[:, sl])
                nc.sync.dma_start(goh[:], goh_d[:])
                for k in range(4):
                    nc.sync.dma_start(wihcT[k][:], wihcT_d[k * 128:(k + 1) * 128, :])
                for k in range(2):
                    nc.sync.dma_start(whhT[k][:], whhT_d[k * 128:(k + 1) * 128, :])
                    nc.sync.dma_start(wgenT[k][:], wgenT_d[k * 128:(k + 1) * 128, :])
                nc.sync.dma_start(bgen[:], bgen_d[:])

                for m in range(2):
                    for nb in range(BL * T // 512):
                        ps = spp.tile([128, 512], dt.float32, tag="hps", name="hps")
                        for k in range(4):
                            nc.tensor.matmul(
                                ps[:],
                                wi2hT[k][:, m * 128:(m + 1) * 128],
                                bhd[k][:, nb * 512:(nb + 1) * 512],
                                start=(k == 0),
                                stop=(k == 3),
                            )
                        nc.vector.tensor_scalar_add(
                            hp[m][:, nb * 512:(nb + 1) * 512], ps[:], bh2h[:, m:m + 1]
                        )

            # ---------- recurrent steps ----------
            with (
                tc.tile_pool(name="work", bufs=1) as wk,
                tc.tile_pool(name="small", bufs=2) as sm,
                tc.tile_pool(name="hidp", bufs=2) as hidp,
                tc.tile_pool(name="ps", bufs=1, space=MemorySpace.PSUM) as pp,
            ):
                u = [None, None]
                hT = [None, None]
                hTb = [None, None]
                for h in range(2):
                    t_f = hidp.tile([128, 2 * HB], dt.float32, tag=f"hT{h}", name=f"hT{h}")
                    t_b = hidp.tile([128, 2 * HB], dt.bfloat16, tag=f"hTb{h}", name=f"hTb{h}")
                    nc.vector.memset(t_f[:], 0.0)
                    nc.vector.memset(t_b[:], 0.0)
                    hT[h] = t_f
                    hTb[h] = t_b

                goh_v = goh[:].rearrange("p (s c b) -> p s c b", c=6, b=BL)
                alphaT = [None, None]

                hp_ps_l = [None, None]

                def prepH(h):
                    # h_proj = W_h2h^T h  (b_h2h is folded into Hp)
                    hp_ps = pp.tile([128, 2 * HB], dt.float32, tag=f"hp_ps{h}", name=f"hp_ps{h}")
                    hp_ps_l[h] = hp_ps
                    for c in range(2):
                        for k in range(2):
                            nc.tensor.matmul(
                                hp_ps[:, c * HB:(c + 1) * HB],
                                wh2hT[k][:, c * 128:(c + 1) * 128],
                                hTb[h][:, k * HB:(k + 1) * HB],
                                start=(c == 0 and k == 0),
                                stop=(c == 1 and k == 1),
                                skip_group_check=True,
                            )
                    u[h] = wk.tile([128, HB * 2 * T], dt.bfloat16, tag=f"u{h}", name=f"u{h}")

                def prepAdd(h, piece):
                    # u[b, c, t] = Hp[c][b, t] + h_proj[c, b]  (broadcast over t)
                    hp_ps = hp_ps_l[h]
                    u_v = u[h][:].rearrange("p (b ct) -> p b ct", ct=2 * T)
                    bsl = slice(h * HB + piece * QB, h * HB + (piece + 1) * QB)
                    osl = slice(piece * QB, (piece + 1) * QB)
                    for c in range(2):
                        nc.vector.tensor_add(
                            u_v[:, osl, c * T:(c + 1) * T],
                            hp[c][:].rearrange("p (b t) -> p b t", t=T)[:, bsl, :],
                            hp_ps[:, c * HB + piece * QB:c * HB + (piece + 1) * QB].to_broadcast((128, QB, T)),
                        )

                def prepTanh(h, piece):
                    sl = slice(piece * QB * 2 * T, (piece + 1) * QB * 2 * T)
                    nc.scalar.activation(u[h][:, sl], u[h][:, sl], AF.Tanh)

                def attn(h):
                    # e_ss packs e (cols 0:2QB) and sum-of-exp (cols 2QB:4QB)
                    e_ss = pp.tile([128, 4 * QB], dt.float32, tag=f"e_ss{h}", name=f"e_ss{h}")
                    for piece in range(2):
                        for c in range(2):
                            for b in range(QB):
                                bl = piece * QB + b
                                nc.tensor.matmul(
                                    e_ss[:, bl:bl + 1],
                                    u[h][:, (bl * 2 + c) * T:(bl * 2 + c + 1) * T],
                                    wsc[:, c:c + 1],
                                    start=(c == 0 and b == 0),
                                    stop=(c == 1 and b == QB - 1),
                                    skip_group_check=True,
                                )
                    expe = sm.tile([128, 2 * QB], dt.bfloat16, tag=f"expe{h}", name=f"expe{h}")
                    nc.scalar.activation(expe[:], e_ss[:, 0:2 * QB], AF.Exp)
                    nc.tensor.matmul(
                        e_ss[:, 2 * QB:4 * QB],
                        ones[:],
                        expe[:],
                        start=True,
                        stop=True,
                        skip_group_check=True,
                    )
                    rs = sm.tile([128, 2 * QB], dt.float32, tag=f"rs{h}", name=f"rs{h}")
                    nc.vector.reciprocal(rs[:], e_ss[:, 2 * QB:4 * QB])
                    al = sm.tile([128, 2 * QB], dt.bfloat16, tag=f"al{h}", name=f"al{h}")
                    nc.vector.tensor_mul(al[:], expe[:], rs[:])
                    alphaT[h] = al
                    ctxT = [None, None]
                    for piece in range(2):
                        ctx_ps = pp.tile([128, 4 * QB], dt.float32, tag="ctx_ps", name=f"ctx_ps{h}{piece}", bufs=2)
                        for ck in range(4):
                            for b in range(QB):
                                gb = h * HB + piece * QB + b
                                nc.tensor.matmul(
                                    ctx_ps[:, ck * QB + b:ck * QB + b + 1],
                                    bht[:, gb * D + ck * 128:gb * D + (ck + 1) * 128],
                                    al[:, piece * QB + b:piece * QB + b + 1],
                                    start=(ck == 0 and b == 0),
                                    stop=(ck == 3 and b == QB - 1),
                                    skip_group_check=True,
                                )
                        ct = sm.tile([128, 4 * QB], dt.bfloat16, tag=f"ctxT{h}{piece}", name=f"ctxT{h}{piece}")
                        nc.scalar.copy(ct[:], ctx_ps[:])
                        ctxT[piece] = ct
                    return ctxT

                def gru(h, s, ctxT):
                    # gi cols 0:6HB, ghn cols 6HB:8HB packed in one psum tile
                    gi_ps = pp.tile([128, 8 * HB], dt.float32, tag="gi_ps", name="gi_ps")
                    for ck in range(4):
                        for m in range(6):
                            for piece in range(2):
                                nc.tensor.matmul(
                                    gi_ps[:, m * HB + piece * QB:m * HB + (piece + 1) * QB],
                                    wihcT[ck][:, m * 128:(m + 1) * 128],
                                    ctxT[piece][:, ck * QB:(ck + 1) * QB],
                                    start=(ck == 0 and m == 0 and piece == 0),
                                    stop=False,
                                    skip_group_check=True,
                                )
                    for m in range(6):
                        nc.tensor.matmul(
                            gi_ps[:, m * HB:(m + 1) * HB],
                            ident[:],
                            goh_v[:, s, m, h * HB:(h + 1) * HB],
                            start=False,
                            stop=False,
                            skip_group_check=True,
                        )
                    for k in range(2):
                        for m in range(4):
                            nc.tensor.matmul(
                                gi_ps[:, m * HB:(m + 1) * HB],
                                whhT[k][:, m * 128:(m + 1) * 128],
                                hTb[h][:, k * HB:(k + 1) * HB],
                                start=False,
                                stop=(k == 1 and m == 3),
                                skip_group_check=True,
                            )
                    for k in range(2):
                        for m in range(4, 6):
                            nc.tensor.matmul(
                                gi_ps[:, (m + 2) * HB:(m + 3) * HB],
                                whhT[k][:, m * 128:(m + 1) * 128],
                                hTb[h][:, k * HB:(k + 1) * HB],
                                start=(k == 0 and m == 4),
                                stop=(k == 1 and m == 5),
                                skip_group_check=True,
                            )
                    # sigmoid(x) = 0.5*tanh(x/2)+0.5; whhT n-cols pre-halved on host
                    trz = sm.tile([128, 4 * HB], dt.float32, tag="trz", name="trz")
                    nc.scalar.activation(trz[:], gi_ps[:, 0:4 * HB], AF.Tanh, scale=0.5)
                    rh = sm.tile([128, 2 * HB], dt.float32, tag="rh", name="rh")
                    nc.vector.scalar_tensor_tensor(
                        rh[:], trz[:, 0:2 * HB], 1.0, gi_ps[:, 6 * HB:8 * HB],
                        op0=mybir.AluOpType.add, op1=mybir.AluOpType.mult,
                    )
                    pre_n = sm.tile([128, 2 * HB], dt.float32, tag="pre_n", name="pre_n")
                    nc.vector.tensor_add(pre_n[:], gi_ps[:, 4 * HB:6 * HB], rh[:])
                    nt = sm.tile([128, 2 * HB], dt.float32, tag="nt", name="nt")
                    nc.scalar.activation(nt[:], pre_n[:], AF.Tanh)
                    dmn = sm.tile([128, 2 * HB], dt.float32, tag="dmn", name="dmn")
                    nc.vector.tensor_sub(dmn[:], hT[h][:], nt[:])
                    zd = sm.tile([128, 2 * HB], dt.float32, tag="zd", name="zd")
                    nc.vector.scalar_tensor_tensor(
                        zd[:], trz[:, 2 * HB:4 * HB], 1.0, dmn[:],
                        op0=mybir.AluOpType.add, op1=mybir.AluOpType.mult,
                    )
                    nh = hidp.tile([128, 2 * HB], dt.float32, tag=f"hT{h}", name=f"hT{h}")
                    nc.vector.scalar_tensor_tensor(
                        nh[:], zd[:], 0.5, nt[:],
                        op0=mybir.AluOpType.mult, op1=mybir.AluOpType.add,
                    )
                    nhb = hidp.tile([128, 2 * HB], dt.bfloat16, tag=f"hTb{h}", name=f"hTb{h}")
                    nc.vector.tensor_copy(nhb[:], nh[:])
                    hT[h] = nh
                    hTb[h] = nhb

                    pr_ps = pp.tile([C, HB], dt.float32, tag="pr_ps", name="pr_ps")
                    for k in range(2):
                        nc.tensor.matmul(
                            pr_ps[:],
                            wgenT[k][:],
                            nhb[:, k * HB:(k + 1) * HB],
                            start=(k == 0),
                            stop=(k == 1),
                            skip_group_check=True,
                        )
                    nc.vector.tensor_add(
                        pacc[:, s * BL + h * HB:s * BL + (h + 1) * HB],
                        pr_ps[:],
                        bgen[:, 0:HB],
                    )

                for h in range(2):
                    prepH(h)
                    for piece in range(2):
                        prepAdd(h, piece)
                        prepTanh(h, piece)
                pend1 = False
                for s in range(NSTEP):
                    last = s + 1 >= NSTEP
                    ctx0 = attn(0)
                    if pend1:
                        # leftover h1 prep piece from previous step
                        prepAdd(1, 1)
                        prepTanh(1, 1)
                        pend1 = False
                    gru(0, s, ctx0)
                    if not last:
                        prepH(0)
                        prepAdd(0, 0)
                        prepTanh(0, 0)
                    ctx1 = attn(1)
                    if not last:
                        prepAdd(0, 1)
                        prepTanh(0, 1)
                    gru(1, s, ctx1)
                    if not last:
                        prepH(1)
                        prepAdd(1, 0)
                        prepTanh(1, 0)
                        if s + 2 >= NSTEP:
                            prepAdd(1, 1)
                            prepTanh(1, 1)
                        else:
                            pend1 = True

            for j in range(4):
                sl = slice(j * NSTEP * BL // 4, (j + 1) * NSTEP * BL // 4)
                nc.sync.dma_start(out_d[:, sl], pacc[:, sl])

    nc.compile()
    return nc


def kernel(**inputs):
    global LAST_RESULT
    from concourse.bass_utils import run_bass_kernel_spmd

    if "nc" not in _CACHE:
        _CACHE["nc"] = _build()
    nc = _CACHE["nc"]

    batch_H = np.asarray(inputs["batch_H"], dtype=np.float32)
    text = np.asarray(inputs["text"])
    W_i2h = np.asarray(inputs["W_i2h"], dtype=np.float32)
    W_h2h = np.asarray(inputs["W_h2h"], dtype=np.float32)
    b_h2h = np.asarray(inputs["b_h2h"], dtype=np.float32)
    W_score = np.asarray(inputs["W_score"], dtype=np.float32)
    W_ih = np.asarray(inputs["W_ih"], dtype=np.float32)
    W_hh = np.asarray(inputs["W_hh"], dtype=np.float32)
    b_ih = np.asarray(inputs["b_ih"], dtype=np.float32)
    b_hh = np.asarray(inputs["b_hh"], dtype=np.float32)
    W_gen = np.asarray(inputs["W_gen"], dtype=np.float32)
    b_gen = np.asarray(inputs["b_gen"], dtype=np.float32)

    shared = {
        "wi2hT": np.ascontiguousarray(W_i2h.T).astype(BF16),
        "wh2hT": np.ascontiguousarray(W_h2h.T).astype(BF16),
        "bh2h": np.ascontiguousarray(b_h2h.reshape(2, 128).T).astype(np.float32),
        "wsc": np.ascontiguousarray(W_score[0].reshape(2, 128).T).astype(BF16),
        "wihcT": np.ascontiguousarray(W_ih[:, :D].T).astype(BF16),
        "whhT": np.ascontiguousarray(W_hh.T * np.concatenate([np.ones(512, np.float32), np.full(256, 0.5, np.float32)])[None, :]).astype(BF16),
        "wgenT": np.ascontiguousarray(W_gen.T).astype(BF16),
        "bgen": np.ascontiguousarray(np.tile(b_gen[:, None], (1, BL))).astype(np.float32),
        "ident": np.eye(128, dtype=np.float32).astype(BF16),
        "ones": np.ones((128, 128), dtype=np.float32).astype(BF16),
    }

    Eoh = W_ih[:, D:]  # [768, 96]
    bias = (b_ih + b_hh)[:, None, None]  # folded; b_hh==0 in this problem

    in_maps = []
    for ci in range(NCORES):
        sh = batch_H[ci * BL:(ci + 1) * BL]  # [64, 128, 512]
        tx = np.asarray(text[ci * BL:(ci + 1) * BL, :NSTEP], dtype=np.int64)  # [64, S]
        A = Eoh[:, tx] + bias  # [768, 64, S]
        gohm = (
            A.reshape(6, 128, BL, NSTEP)
            .transpose(1, 3, 0, 2)
            .reshape(128, NSTEP * 6 * BL)
        )
        m = dict(shared)
        m["bht"] = np.ascontiguousarray(sh.transpose(1, 0, 2).reshape(128, BL * D)).astype(BF16)
        m["bhd"] = np.ascontiguousarray(sh.transpose(2, 0, 1).reshape(D, BL * T)).astype(BF16)
        m["goh"] = np.ascontiguousarray(gohm).astype(BF16)
        in_maps.append(m)

    trace = bool(os.environ.get("ATT_TRACE"))
    res = run_bass_kernel_spmd(nc, in_maps, list(range(NCORES)), trace=trace)
    LAST_RESULT = res

    outs = []
    for r in res.results:
        o = r["out"].reshape(C, NSTEP, BL).transpose(2, 1, 0)  # [64, S, 96]
        outs.append(o)
    return np.ascontiguousarray(np.concatenate(outs, axis=0)).astype(np.float32)
